# revision 15
# baseline (speedup 1.0000x reference)
"""Bi-Real-Net BasicBlock (binary activation + binarized 3x3 conv + BN + residual)
as an 8-core Trainium2 Bass kernel.

Strategy: data-parallel over batch (8 images per core). Forward values:
  a  = sign(x)                      (exact +-1, fp8e4)
  wb = scale[o] * sign(w)           (scale applied at PSUM evacuation)
  y  = conv3x3(a, sign(w))          (fp8 DoubleRow matmuls: 9 taps accumulate
                                     into each PSUM tile, contraction over all
                                     256 input channels per matmul; implicit
                                     im2col on a zero-padded 30x30 layout)
  BN uses exact global batch stats: per-core per-channel sum/sumsq are
  AllGather-ed across the 8 cores and reduced on-chip; then
  out = y * (gamma*rsqrt(var+eps)) + (beta - mean*gamma*rsqrt(var+eps)) + x.

All values flowing into the matmul are exactly representable (+-1, 0) and PSUM
accumulates in fp32, so the conv is bit-exact; BN stats match the reference to
fp32 rounding. Weights are host-prepped (sign + per-channel scale + lhsT
transpose) since they are static layer parameters; all activation math runs on
device.
"""
import os

os.environ.setdefault("BASS_NEVER_TRACE", "1")

import numpy as np

N_CORES = 8
B = 8            # images per core
C = 256          # channels (in == out)
H = W = 28
HP = WP = 30     # zero-padded image
IMG = HP * WP    # 900
GUARD = 32       # zero guard before/after the padded batch strip
ASZ = GUARD + B * IMG + GUARD   # 7264 free elems per ic chunk (16-aligned)
NTOT = 64 * H * W               # BN normalization count (full batch)
EPS = 1e-5

_CACHE = {}


def _build_nc(collective=True):
    import concourse.bacc as bacc
    import concourse.mybir as mybir
    import concourse.tile as tile

    f32 = mybir.dt.float32
    fp8 = mybir.dt.float8e4
    ALU = mybir.AluOpType
    ACT = mybir.ActivationFunctionType

    nc = bacc.Bacc("TRN2", target_bir_lowering=False, debug=False,
                   enable_asserts=True,
                   num_devices=N_CORES if collective else 1)
    x_d = nc.dram_tensor("x", [B, C, H, W], f32, kind="ExternalInput")
    # host-packed lhsT weights: [i, (tap, oc, ic), o] = sign(w)[oc*128+o, ic*128+i, tap]
    w_d = nc.dram_tensor("wls", [128, 9, 2, 2, 128], fp8, kind="ExternalInput")
    # host-packed per-channel params: cols = scale(oc0), scale(oc1), gamma(oc0),
    # gamma(oc1), beta(oc0), beta(oc1)
    s_d = nc.dram_tensor("sgb", [128, 6], f32, kind="ExternalInput")
    o_d = nc.dram_tensor("out", [B, C, H, W], f32, kind="ExternalOutput")

    with tile.TileContext(nc) as tc:
        with (
            tc.tile_pool(name="persist", bufs=1) as pp,
            tc.tile_pool(name="scratch", bufs=3) as sp,
            tc.tile_pool(name="psum", bufs=8, space="PSUM") as psp,
            tc.tile_pool(name="dram", bufs=1, space="DRAM") as dp,
        ):
            # constant APs used as activation biases (sign needs 0.0, BN sqrt
            # uses eps)
            zc = pp.tile([128, 1], f32, tag="zc", name="zc")
            ec = pp.tile([128, 1], f32, tag="ec", name="ec")
            nc.vector.memset(zc[:], 0.0)
            nc.vector.memset(ec[:], EPS)
            nc.const_aps.aps[(f32, 0.0)] = zc[:]
            nc.const_aps.aps[(f32, EPS)] = ec[:]

            wsb = pp.tile([128, 9, 2, 2, 128], fp8, tag="wsb", name="wsb")
            sgb = pp.tile([128, 6], f32, tag="sgb", name="sgb")
            nc.sync.dma_start(wsb[:], w_d[:])
            nc.sync.dma_start(sgb[:], s_d[:])

            x_sb = [pp.tile([128, B, H, W], f32, tag=f"x{ic}", name=f"x{ic}")
                    for ic in range(2)]
            y_sb = [pp.tile([128, B, H, W], f32, tag=f"y{oc}", name=f"y{oc}")
                    for oc in range(2)]
            # both ic chunks of sign(x) in one tile so a DoubleRow matmul can
            # read them as one 3D AP [128, 2, N]
            a8 = pp.tile([128, 2, ASZ], fp8, tag="a8", name="a8")

            # zero only what matmuls read: the guards and each image's one-pixel
            # pad frame (interiors are fully overwritten by sign()).
            nc.vector.memset(a8[:, :, :GUARD], 0.0)
            nc.vector.memset(a8[:, :, ASZ - GUARD:], 0.0)
            for n in range(B):
                img = a8[:, :, GUARD + n * IMG: GUARD + (n + 1) * IMG] \
                    .rearrange("p i (r c) -> p i r c", c=WP)
                nc.vector.memset(img[:, :, 0, :], 0.0)
                nc.vector.memset(img[:, :, HP - 1, :], 0.0)
                nc.vector.memset(img[:, :, 1:HP - 1, 0:1], 0.0)
                nc.vector.memset(img[:, :, 1:HP - 1, WP - 1:WP], 0.0)

            for n in range(B):
                for ic in range(2):
                    nc.sync.dma_start(x_sb[ic][:, n], x_d[n, ic * 128:(ic + 1) * 128])
                    # sign(x) into the padded interior (rows/cols 1..28 of 30x30)
                    off = GUARD + n * IMG + HP + 1
                    dst = a8[:, ic, off:off + 28 * HP] \
                        .rearrange("p (h w) -> p h w", w=HP)[:, :, :W]
                    nc.scalar.sign(dst, x_sb[ic][:, n])

            # conv: per output-channel chunk, per group of 4 PSUM chunks
            # (= 2 images): 9 DoubleRow matmuls accumulate into each PSUM tile,
            # then ACT evacuates (applies scale, accumulates sum) while DVE
            # squares the raw PSUM for sumsq.
            sums = [pp.tile([128, 16], f32, tag=f"sums{oc}", name=f"sums{oc}")
                    for oc in range(2)]
            ssqs = [pp.tile([128, 16], f32, tag=f"ssqs{oc}", name=f"ssqs{oc}")
                    for oc in range(2)]
            for oc in range(2):
                for g in range(4):
                    pt = [psp.tile([128, 14, HP], f32, tag="pt", name="pt")
                          for _ in range(4)]
                    for t in range(9):
                        dh, dw = t // 3, t % 3
                        lhs = wsb[:, t, oc]
                        for bi in range(4):
                            img, half = g * 2 + bi // 2, bi % 2
                            off = (GUARD + img * IMG + HP + half * 420
                                   + (dh - 1) * HP + (dw - 1))
                            nc.tensor.matmul(
                                pt[bi][:], lhs, a8[:, :, off:off + 420],
                                start=(t == 0), stop=(t == 8),
                                perf_mode=mybir.MatmulPerfMode.DoubleRow)
                    for bi in range(4):
                        img, half = g * 2 + bi // 2, bi % 2
                        idx = g * 4 + bi
                        dst = y_sb[oc][:, img, half * 14:(half + 1) * 14, :]
                        nc.vector.tensor_scalar(
                            dst, pt[bi][:, :, 1:29], sgb[:, oc:oc + 1], 0.0,
                            ALU.mult, ALU.add,
                            accum_out=sums[oc][:, idx:idx + 1])
                        sq = sp.tile([128, 14, W], f32, tag="sq", name="sq")
                        nc.scalar.activation(
                            sq[:], dst, ACT.Square,
                            accum_out=ssqs[oc][:, idx:idx + 1])

            # pack per-core stats [S1(oc0), S1(oc1), S2raw(oc0), S2raw(oc1)]
            stat_in = pp.tile([128, 4], f32, tag="stat_in", name="stat_in")
            for oc in range(2):
                nc.vector.reduce_sum(stat_in[:, oc:oc + 1], sums[oc][:],
                                     axis=mybir.AxisListType.X)
                nc.vector.reduce_sum(stat_in[:, 2 + oc:3 + oc], ssqs[oc][:],
                                     axis=mybir.AxisListType.X)

            # cross-core: AllGather the 4 stat columns, reduce locally
            stat_tot = pp.tile([128, 4], f32, tag="stat_tot", name="stat_tot")
            if collective:
                b_in = dp.tile([128, 4], f32, tag="b_in", name="b_in")
                b_out = dp.tile([N_CORES * 128, 4], f32, tag="b_out",
                                name="b_out")
                nc.sync.dma_start(b_in[:], stat_in[:])
                nc.gpsimd.collective_compute(
                    "AllGather", ALU.bypass,
                    ins=[b_in.opt()], outs=[b_out.opt()],
                    replica_groups=[list(range(N_CORES))])
                stat_all = pp.tile([128, 4, N_CORES], f32, tag="stat_all",
                                   name="stat_all")
                nc.sync.dma_start(
                    stat_all[:], b_out.rearrange("(c p) k -> p k c", p=128))
                nc.vector.reduce_sum(stat_tot[:], stat_all[:],
                                     axis=mybir.AxisListType.X)
            else:
                # single-core cost-model variant: stats scaled as if global
                nc.vector.tensor_scalar(stat_tot[:], stat_in[:],
                                        float(N_CORES), None, ALU.mult)

            # per-channel BN affine:  k = gamma / sqrt(var+eps),  b = beta - mean*k
            kb = pp.tile([128, 4], f32, tag="kb", name="kb")
            for oc in range(2):
                mean = sp.tile([128, 1], f32, tag="mean", name="mean")
                e2 = sp.tile([128, 1], f32, tag="e2", name="e2")
                var = sp.tile([128, 1], f32, tag="var", name="var")
                sd = sp.tile([128, 1], f32, tag="sd", name="sd")
                nc.vector.tensor_scalar(mean[:], stat_tot[:, oc:oc + 1],
                                        1.0 / NTOT, None, ALU.mult)
                nc.vector.tensor_scalar(e2[:], stat_tot[:, 2 + oc:3 + oc],
                                        1.0 / NTOT, None, ALU.mult)
                msq = sp.tile([128, 1], f32, tag="msq", name="msq")
                nc.vector.tensor_tensor(msq[:], mean[:], mean[:], ALU.mult)
                nc.vector.tensor_tensor(var[:], e2[:], msq[:], ALU.subtract)
                nc.scalar.activation(sd[:], var[:], ACT.Sqrt, bias=EPS)
                inv = sp.tile([128, 1], f32, tag="inv", name="inv")
                nc.vector.reciprocal(inv[:], sd[:])
                nc.vector.tensor_tensor(kb[:, oc:oc + 1], inv[:],
                                        sgb[:, 2 + oc:3 + oc], ALU.mult)
                mk = sp.tile([128, 1], f32, tag="mk", name="mk")
                nc.vector.tensor_tensor(mk[:], mean[:], kb[:, oc:oc + 1],
                                        ALU.mult)
                nc.vector.tensor_tensor(kb[:, 2 + oc:3 + oc],
                                        sgb[:, 4 + oc:5 + oc], mk[:],
                                        ALU.subtract)

            # apply: out = y*k + (x + b), per image, streamed out
            for oc in range(2):
                for n in range(B):
                    xpb = sp.tile([128, H, W], f32, tag="xpb", name="xpb")
                    nc.scalar.activation(xpb[:], x_sb[oc][:, n], ACT.Identity,
                                         bias=kb[:, 2 + oc:3 + oc])
                    ot = sp.tile([128, H, W], f32, tag="ot", name="ot")
                    nc.vector.scalar_tensor_tensor(
                        ot[:], y_sb[oc][:, n], kb[:, oc:oc + 1], xpb[:],
                        ALU.mult, ALU.add)
                    nc.sync.dma_start(o_d[n, oc * 128:(oc + 1) * 128], ot[:])

    nc.compile()
    return nc


def _prep_weights(weights, gamma, beta):
    import concourse.mybir as mybir
    fp8_np = mybir.dt.np(mybir.dt.float8e4)
    w = np.asarray(weights, dtype=np.float32).reshape(C, C, 9)
    scale = np.mean(np.abs(w), axis=(1, 2), dtype=np.float32)
    ws = np.sign(w).reshape(2, 128, 2, 128, 9)        # [ocb, o, icb, i, t]
    wls = np.ascontiguousarray(
        ws.transpose(3, 4, 0, 2, 1)                   # [i, t, ocb, icb, o]
    ).astype(fp8_np)
    g = np.asarray(gamma, dtype=np.float32)
    bt = np.asarray(beta, dtype=np.float32)
    sgb = np.stack([scale[:128], scale[128:], g[:128], g[128:],
                    bt[:128], bt[128:]], axis=1).astype(np.float32)
    return np.ascontiguousarray(wls), np.ascontiguousarray(sgb)


def _make_runner(nc):
    """Cached variant of bass2jax.run_bass_via_pjrt's multi-core path: the
    jitted shard_map is built once, so repeat kernel() calls skip re-tracing."""
    import jax
    import concourse.mybir as mybir
    from concourse import bass2jax
    from jax.experimental.shard_map import shard_map
    from jax.sharding import Mesh, PartitionSpec

    bass2jax.install_neuronx_cc_hook()
    partition_name = (nc.partition_id_tensor.name
                      if nc.partition_id_tensor else None)

    in_names, out_names, out_avals, zero_outs = [], [], [], []
    for alloc in nc.m.functions[0].allocations:
        if not isinstance(alloc, mybir.MemoryLocationSet):
            continue
        name = alloc.memorylocations[0].name
        if alloc.kind == "ExternalInput":
            if name != partition_name:
                in_names.append(name)
        elif alloc.kind == "ExternalOutput":
            out_names.append(name)
            shape = tuple(alloc.tensor_shape)
            dtype = mybir.dt.np(alloc.dtype)
            out_avals.append(jax.core.ShapedArray(shape, dtype))
            zero_outs.append(np.zeros(shape, dtype))
    n_params = len(in_names)
    n_outs = len(out_avals)
    all_in_names = tuple(in_names + out_names + (
        [partition_name] if partition_name else []))
    donate = tuple(range(n_params, n_params + n_outs))

    def _body(*args):
        operands = list(args)
        if partition_name is not None:
            operands.append(bass2jax.partition_id_tensor())
        return tuple(bass2jax._bass_exec_p.bind(
            *operands,
            out_avals=tuple(out_avals),
            in_names=all_in_names,
            out_names=tuple(out_names),
            lowering_input_output_aliases=(),
            sim_require_finite=True,
            sim_require_nnan=True,
            nc=nc,
        ))

    devices = jax.devices()[:N_CORES]
    mesh = Mesh(np.asarray(devices), ("core",))
    in_specs = (PartitionSpec("core"),) * (n_params + n_outs)
    out_specs = (PartitionSpec("core"),) * n_outs
    sharded = jax.jit(
        shard_map(_body, mesh=mesh, in_specs=in_specs, out_specs=out_specs,
                  check_rep=False),
        donate_argnums=donate, keep_unused=True)

    def run(per_core_inputs):
        concat_in = [
            np.concatenate([m[name] for m in per_core_inputs], axis=0)
            for name in in_names
        ]
        concat_zeros = [
            np.zeros((N_CORES * z.shape[0], *z.shape[1:]), z.dtype)
            for z in zero_outs
        ]
        out_arrs = sharded(*concat_in, *concat_zeros)
        return {name: np.asarray(out_arrs[i]) for i, name in enumerate(out_names)}

    return run


def kernel(x, weights, gamma, beta):
    if "run" not in _CACHE:
        _CACHE["run"] = _make_runner(_build_nc())
    x = np.asarray(x, dtype=np.float32)
    wls, sgb = _prep_weights(weights, gamma, beta)
    in_maps = [
        {"x": np.ascontiguousarray(x[c * B:(c + 1) * B]), "wls": wls, "sgb": sgb}
        for c in range(N_CORES)
    ]
    outs = _CACHE["run"](in_maps)
    return outs["out"].reshape(64, C, H, W)
